# revision 25
# baseline (speedup 1.0000x reference)
"""Trainium2 Bass kernel for nn_CELoss_4896262717859.

Computes, for each query column c = idx_node[k] of a sparse adjacency matrix
(diagonal zeroed), a cross-entropy-style loss over the "lower" (r < c) and
"upper" (r > c) neighbor sets:

    contrib_side(c) = [cnt>0 and poscnt==1] * (log(sum_r m exp(out_r)) - poslogit) / cnt

All per-column quantities are sums of the form sum_r adj[r,c] * w[r] for
w in {1, pos, pos*out, exp(out)} -> computed as tensor-engine matvecs with a
triangular split, per-column for ALL N columns, then gathered at idx_node on
the host (O(N+K) combine).

Sharding: columns split into 8 slabs of 1024 (one per core). Each core reads
its [8192 x 1024] int32 slab contiguously (memory roofline), casts to bf16,
and accumulates psum[12, 1024] stats = {L,U} x {ones, pos, pl_hi, pl_lo,
e_hi, e_lo}. The core's row order is rotated by 1024*core so the diagonal
block always falls in local row-tiles 0..7 -> one NEFF serves all cores; the
L/U routing of full tiles is data-driven via zero-padded weight variants.
"""

import numpy as np
import ml_dtypes

N = 8192
K = 4096
NCORES = 8
SLAB = N // NCORES        # 1024 columns per core
P = 128                   # partition / tile edge
NT = N // P               # 64 row tiles
TPC = SLAB // P           # 8 diagonal tiles per core
NW = 6                    # weights per side
M = 2 * NW                # 12 psum partitions (L half = 0:6, U half = 6:12)
MMN = 512                 # max matmul free size

BF16 = ml_dtypes.bfloat16

_BASS_CACHE = {}


def _build_bass():
    import concourse.tile as tile
    import concourse.mybir as mybir
    from concourse import bacc

    # Bacc (not raw Bass): its compile() runs generate_event_semaphores,
    # which splits multi-sem waits — TRN2 instructions hold at most one.
    nc = bacc.Bacc("TRN2")
    adj = nc.dram_tensor("adj", [N, SLAB], mybir.dt.int32, kind="ExternalInput")
    wmat = nc.dram_tensor(
        "wmat", [P, (NT + TPC) * M], mybir.dt.bfloat16, kind="ExternalInput"
    )
    masks = nc.dram_tensor("masks", [P, 2 * P], mybir.dt.bfloat16, kind="ExternalInput")
    stats = nc.dram_tensor("stats", [M, SLAB], mybir.dt.float32, kind="ExternalOutput")

    with tile.TileContext(nc) as tc:
        with (
            tc.tile_pool(name="singles", bufs=1) as singles,
            # bufs multiple of 8 matches the 8-queue HWDGE round-robin: the
            # slot-reuse predecessor of each adj DMA lands on the SAME queue,
            # so its WAW ordering is implicit and the DMA carries a single
            # sync-wait (the DMA ISA struct has room for only one).
            tc.tile_pool(name="io", bufs=8) as io_pool,
            tc.tile_pool(name="bf", bufs=6) as bf_pool,
            tc.tile_pool(name="diag", bufs=TPC) as diag_pool,
            tc.tile_pool(name="psum", bufs=1, space="PSUM") as psum_pool,
        ):
            # issue the first two adjacency DMAs before anything else so the
            # HBM-saturated stream (the critical path) starts ~1.3us earlier;
            # the small wmat/masks loads slot in behind them.
            pre = {}
            for j in range(2):
                t = io_pool.tile([P, SLAB], mybir.dt.int32, tag="adj_i")
                nc.sync.dma_start(out=t, in_=adj[j * P : (j + 1) * P, :])
                pre[j] = t

            wsb = singles.tile([P, (NT + TPC) * M], mybir.dt.bfloat16)
            nc.sync.dma_start(out=wsb, in_=wmat[:, :])
            msb_raw = singles.tile([P, 2 * P], mybir.dt.bfloat16)
            nc.sync.dma_start(out=msb_raw, in_=masks[:, :])
            # Re-produce the masks on DVE: the DVE TensorTensor ISA struct has
            # room for a single sync-wait, so the diag-mask multiplies must
            # only ever depend on DVE-produced operands (one self-sem wait).
            msb = singles.tile([P, 2 * P], mybir.dt.bfloat16)
            nc.vector.tensor_copy(msb, msb_raw)

            acc = psum_pool.tile([M, SLAB], mybir.dt.float32)

            def wv(v):
                return wsb[:, v * M : (v + 1) * M]

            # start=True zeroes the ENTIRE psum bank(s) a matmul touches, so
            # (a) every matmul stays inside one 512-col bank, (b) exactly the
            # first matmul touching each bank carries start=True.
            bank_started = [False] * (SLAB // MMN)

            def mm_seg(w, rhs_slice, a, b, stop=False):
                bank = a // MMN
                assert b <= (bank + 1) * MMN
                nc.tensor.matmul(
                    acc[:, a:b], w, rhs_slice,
                    start=not bank_started[bank], stop=stop,
                    skip_group_check=True,
                )
                bank_started[bank] = True

            def mm(w, rhs_full, a, b, stop=False):
                while a < b:
                    e = min(b, (a // MMN + 1) * MMN)
                    mm_seg(w, rhs_full[:, a:e], a, e, stop=stop)
                    a = e

            for j in range(NT):
                last = j == NT - 1
                if j in pre:
                    adj_i = pre.pop(j)
                else:
                    adj_i = io_pool.tile([P, SLAB], mybir.dt.int32, tag="adj_i")
                    if last:
                        # split the final load so its first half (and the
                        # bank-A matmul) overlaps the second half's transfer
                        nc.sync.dma_start(
                            out=adj_i[:, 0:MMN], in_=adj[j * P :, 0:MMN]
                        )
                        nc.sync.dma_start(
                            out=adj_i[:, MMN:], in_=adj[j * P :, MMN:]
                        )
                    else:
                        nc.sync.dma_start(out=adj_i, in_=adj[j * P : (j + 1) * P, :])
                adj_b = bf_pool.tile([P, SLAB], mybir.dt.bfloat16)
                if last:
                    nc.vector.tensor_copy(adj_b[:, 0:MMN], adj_i[:, 0:MMN])
                    mm(wv(j), adj_b, 0, MMN)
                    nc.vector.tensor_copy(adj_b[:, MMN:], adj_i[:, MMN:])
                    mm(wv(j), adj_b, MMN, SLAB, stop=True)
                    continue
                nc.vector.tensor_copy(adj_b, adj_i)

                if j < TPC:
                    WL, WU = wv(j), wv(NT + j)
                    c0, c1 = j * P, (j + 1) * P
                    mlo = diag_pool.tile([P, P], mybir.dt.bfloat16)
                    nc.vector.tensor_mul(mlo, adj_b[:, c0:c1], msb[:, 0:P])
                    mup = diag_pool.tile([P, P], mybir.dt.bfloat16)
                    nc.vector.tensor_mul(mup, adj_b[:, c0:c1], msb[:, P : 2 * P])
                    # full columns left of the diag block: rows > cols -> U
                    mm(WU, adj_b, 0, c0)
                    mm_seg(WL, mlo, c0, c1)
                    mm_seg(WU, mup, c0, c1)
                    # full columns right of the diag block: rows < cols -> L
                    mm(WL, adj_b, c1, SLAB)
                else:
                    mm(wv(j), adj_b, 0, SLAB, stop=last)

            # per-bank copy-out: bank A's copy/DMA overlap the final bank-B
            # matmul (ACT reads psum bank A while PE writes bank B)
            out_sb = singles.tile([M, SLAB], mybir.dt.float32)
            nc.scalar.copy(out_sb[:, 0:MMN], acc[:, 0:MMN])
            nc.sync.dma_start(out=stats[:, 0:MMN], in_=out_sb[:, 0:MMN])
            nc.scalar.copy(out_sb[:, MMN:], acc[:, MMN:])
            nc.sync.dma_start(out=stats[:, MMN:], in_=out_sb[:, MMN:])

    nc.compile()
    return nc


def _split_bf16(v):
    hi = v.astype(BF16)
    lo = (v - hi.astype(np.float64)).astype(BF16)
    return hi, lo


def _host_prep(outputs, targets):
    """Per-row weight table Wside [N, 6] bf16 and per-core inputs."""
    out = np.asarray(outputs, np.float64).reshape(-1)
    pos = (np.asarray(targets).reshape(-1) != 0).astype(np.float64)
    pl_hi, pl_lo = _split_bf16(pos * out)
    e_hi, e_lo = _split_bf16(np.exp(out))
    wside = np.stack(
        [
            np.ones(N, BF16),
            pos.astype(BF16),
            pl_hi,
            pl_lo,
            e_hi,
            e_lo,
        ],
        axis=1,
    ).astype(BF16)  # [N, 6]

    # triangular masks for the diagonal 128-block (strict)
    ri = np.arange(P)[:, None]
    ci = np.arange(P)[None, :]
    masks = np.concatenate(
        [(ri < ci).astype(BF16), (ri > ci).astype(BF16)], axis=1
    )  # [128, 256]
    return wside, np.ascontiguousarray(masks)


def _build_wmat(wside, core):
    """Per-core weight variants [128, (64+8)*12] bf16.

    Variant j (j<64): weights for local row tile j (absolute tile (8*core+j)%64).
      j < 8  -> L-only variant (diag tiles; U-only twin stored at 64+j)
      j >= 8 -> single variant, L or U half per the tile's position vs the slab
    """
    w = np.zeros((P, NT + TPC, M), dtype=BF16)
    for j in range(NT):
        t = (TPC * core + j) % NT
        rows = wside[t * P : (t + 1) * P, :]  # [128, 6]
        if j < TPC:
            w[:, j, 0:NW] = rows
            w[:, NT + j, NW:M] = rows
        elif j < NT - TPC * core:
            w[:, j, NW:M] = rows  # rows above slab columns -> U
        else:
            w[:, j, 0:NW] = rows  # wrapped rows below slab columns -> L
    return np.ascontiguousarray(w.reshape(P, (NT + TPC) * M))


def _build_shard(node_adj, core):
    """Rotated column slab [N, SLAB] int32: local row rho = (abs_row - SLAB*core) mod N."""
    c0 = SLAB * core
    cols = node_adj[:, c0 : c0 + SLAB]
    if core == 0:
        return np.ascontiguousarray(cols, dtype=np.int32)
    return np.ascontiguousarray(
        np.concatenate([cols[c0:], cols[:c0]], axis=0), dtype=np.int32
    )


def _combine(stats_list, idx_node):
    """stats_list: per-core [12, SLAB] f32 -> scalar loss (f64 math)."""
    full = np.concatenate([np.asarray(s, np.float64) for s in stats_list], axis=1)

    def side_contrib(x):
        cnt, poscnt = x[0], x[1]
        poslogit = x[2] + x[3]
        sumexp = x[4] + x[5]
        valid = (cnt > 0.5) & (np.abs(poscnt - 1.0) < 0.25)
        lse = np.log(np.where(valid, np.maximum(sumexp, 1e-300), 1.0))
        return np.where(valid, (lse - poslogit) / np.maximum(cnt, 1.0), 0.0)

    contrib = side_contrib(full[0:NW]) + side_contrib(full[NW:M])
    idx = np.asarray(idx_node).reshape(-1).astype(np.int64)
    return np.array(contrib[idx].sum(), dtype=np.float32)


def _ensure_axon_hooks_stub():
    """bass_utils imports antenv.axon_hooks when tracing is requested via
    env; the module is absent on some images. Provide a no-op stub so the
    import never crashes (hook=None -> bass_utils skips tracing)."""
    import sys
    import types

    try:
        import antenv.axon_hooks  # noqa: F401
    except ImportError:
        mod = types.ModuleType("antenv.axon_hooks")
        state = {"hook": None}
        mod.set_axon_ntff_profile_hook = lambda h: state.__setitem__("hook", h)
        mod.get_axon_ntff_profile_hook = lambda: state["hook"]
        sys.modules["antenv.axon_hooks"] = mod


def _device_stats(in_maps):
    _ensure_axon_hooks_stub()
    from concourse.bass_utils import run_bass_kernel_spmd

    if "nc" not in _BASS_CACHE:
        _BASS_CACHE["nc"] = _build_bass()
    last_exc = None
    for attempt in range(4):
        try:
            res = run_bass_kernel_spmd(
                _BASS_CACHE["nc"], in_maps, core_ids=list(range(NCORES))
            )
            return [r["stats"] for r in res.results]
        except Exception as e:  # transient NRT/accelerator hiccups
            last_exc = e
            try:
                # a fresh PJRT client usually recovers a transiently
                # "unrecoverable" accelerator; mirrors a process restart
                import jax
                import jax.extend.backend as _jeb

                jax.clear_caches()
                _jeb.clear_backends()
            except Exception:
                pass
            import time

            time.sleep(2.0 * (attempt + 1))
    raise last_exc


def _sim_stats(in_maps):
    """Numpy emulation of the device kernel (same inputs), for logic validation."""
    outs = []
    for m in in_maps:
        adj = m["adj"].astype(np.float32)
        w = m["wmat"].reshape(P, NT + TPC, M).astype(np.float32)
        msk = m["masks"].astype(np.float32)
        lowm, upm = msk[:, 0:P], msk[:, P:]
        acc = np.zeros((M, SLAB), np.float32)
        for j in range(NT):
            tile = adj[j * P : (j + 1) * P, :]
            if j < TPC:
                WL, WU = w[:, j, :], w[:, NT + j, :]
                c0, c1 = j * P, (j + 1) * P
                acc[:, :c0] += WU.T @ tile[:, :c0]
                acc[:, c0:c1] += WL.T @ (tile[:, c0:c1] * lowm)
                acc[:, c0:c1] += WU.T @ (tile[:, c0:c1] * upm)
                acc[:, c1:] += WL.T @ tile[:, c1:]
            else:
                acc += w[:, j, :].T @ tile
        outs.append(acc)
    return outs


def kernel(outputs, targets, node_adj, idx_node, _simulate=False):
    node_adj = np.asarray(node_adj)
    wside, masks = _host_prep(outputs, targets)
    in_maps = [
        {
            "adj": _build_shard(node_adj, d),
            "wmat": _build_wmat(wside, d),
            "masks": masks,
        }
        for d in range(NCORES)
    ]
    stats = _sim_stats(in_maps) if _simulate else _device_stats(in_maps)
    return _combine(stats, idx_node)


# revision 29
# speedup vs baseline: 1.0298x; 1.0298x over previous
"""Trainium2 Bass kernel for nn_CELoss_4896262717859.

Computes, for each query column c = idx_node[k] of a sparse adjacency matrix
(diagonal zeroed), a cross-entropy-style loss over the "lower" (r < c) and
"upper" (r > c) neighbor sets:

    contrib_side(c) = [cnt>0 and poscnt==1] * (log(sum_r m exp(out_r)) - poslogit) / cnt

All per-column quantities are sums of the form sum_r adj[r,c] * w[r] for
w in {1, pos, pos*out, exp(out)} -> computed as tensor-engine matvecs with a
triangular split, per-column for ALL N columns, then gathered at idx_node on
the host (O(N+K) combine).

Sharding: columns split into 8 slabs of 1024 (one per core). Each core reads
its [8192 x 1024] int32 slab contiguously (memory roofline), casts to bf16,
and accumulates psum[12, 1024] stats = {L,U} x {ones, pos, pl_hi, pl_lo,
e_hi, e_lo}. The core's row order is rotated by 1024*core so the diagonal
block always falls in local row-tiles 0..7 -> one NEFF serves all cores; the
L/U routing of full tiles is data-driven via zero-padded weight variants.
"""

import numpy as np
import ml_dtypes

N = 8192
K = 4096
NCORES = 8
SLAB = N // NCORES        # 1024 columns per core
P = 128                   # partition / tile edge
NT = N // P               # 64 row tiles
TPC = SLAB // P           # 8 diagonal tiles per core
NW = 6                    # weights per side
M = 2 * NW                # 12 psum partitions (L half = 0:6, U half = 6:12)
MMN = 512                 # max matmul free size

BF16 = ml_dtypes.bfloat16

_BASS_CACHE = {}


def _build_bass():
    import concourse.tile as tile
    import concourse.mybir as mybir
    from concourse import bacc

    # Bacc (not raw Bass): its compile() runs generate_event_semaphores,
    # which splits multi-sem waits — TRN2 instructions hold at most one.
    nc = bacc.Bacc("TRN2")
    adj = nc.dram_tensor("adj", [N, SLAB], mybir.dt.int32, kind="ExternalInput")
    wmat = nc.dram_tensor(
        "wmat", [P, (NT + TPC) * M], mybir.dt.bfloat16, kind="ExternalInput"
    )
    masks = nc.dram_tensor("masks", [P, 2 * P], mybir.dt.bfloat16, kind="ExternalInput")
    stats = nc.dram_tensor("stats", [M, SLAB], mybir.dt.float32, kind="ExternalOutput")

    with tile.TileContext(nc) as tc:
        with (
            tc.tile_pool(name="singles", bufs=1) as singles,
            # bufs multiple of 8 matches the 8-queue HWDGE round-robin: the
            # slot-reuse predecessor of each adj DMA lands on the SAME queue,
            # so its WAW ordering is implicit and the DMA carries a single
            # sync-wait (the DMA ISA struct has room for only one).
            tc.tile_pool(name="io", bufs=8) as io_pool,
            tc.tile_pool(name="bf", bufs=6) as bf_pool,
            tc.tile_pool(name="diag", bufs=TPC) as diag_pool,
            tc.tile_pool(name="psum", bufs=1, space="PSUM") as psum_pool,
        ):
            # issue the first two adjacency DMAs before anything else so the
            # HBM-saturated stream (the critical path) starts ~1.3us earlier;
            # the small wmat/masks loads slot in behind them.
            pre = {}
            for j in range(2):
                t = io_pool.tile([P, SLAB], mybir.dt.int32, tag="adj_i")
                nc.sync.dma_start(out=t, in_=adj[j * P : (j + 1) * P, :])
                pre[j] = t

            wsb = singles.tile([P, (NT + TPC) * M], mybir.dt.bfloat16)
            nc.sync.dma_start(out=wsb, in_=wmat[:, :])
            msb_raw = singles.tile([P, 2 * P], mybir.dt.bfloat16)
            nc.sync.dma_start(out=msb_raw, in_=masks[:, :])
            # Re-produce the masks on DVE: the DVE TensorTensor ISA struct has
            # room for a single sync-wait, so the diag-mask multiplies must
            # only ever depend on DVE-produced operands (one self-sem wait).
            msb = singles.tile([P, 2 * P], mybir.dt.bfloat16)
            nc.vector.tensor_copy(msb, msb_raw)

            # one psum tile per 512-col bank: Tile's RAW deps are whole-tile,
            # so separate tiles let bank A's copy-out overlap bank B's final
            # matmuls
            accs = [
                psum_pool.tile(
                    [M, MMN], mybir.dt.float32, tag=f"acc{b}", name=f"acc{b}"
                )
                for b in range(SLAB // MMN)
            ]

            def wv(v):
                return wsb[:, v * M : (v + 1) * M]

            # start=True zeroes the ENTIRE psum bank(s) a matmul touches, so
            # (a) every matmul stays inside one 512-col bank, (b) exactly the
            # first matmul touching each bank carries start=True.
            bank_started = [False] * (SLAB // MMN)

            def mm_seg(w, rhs_slice, a, b, stop=False):
                bank = a // MMN
                assert b <= (bank + 1) * MMN
                nc.tensor.matmul(
                    accs[bank][:, a - bank * MMN : b - bank * MMN], w, rhs_slice,
                    start=not bank_started[bank], stop=stop,
                    skip_group_check=True,
                )
                bank_started[bank] = True

            def mm(w, rhs_full, a, b, stop=False):
                while a < b:
                    e = min(b, (a // MMN + 1) * MMN)
                    mm_seg(w, rhs_full[:, a:e], a, e, stop=stop)
                    a = e

            for j in range(NT):
                last = j == NT - 1
                if j in pre:
                    adj_i = pre.pop(j)
                else:
                    adj_i = io_pool.tile([P, SLAB], mybir.dt.int32, tag="adj_i")
                    if last:
                        # split the final load so its first half (and the
                        # bank-A matmul) overlaps the second half's transfer
                        nc.sync.dma_start(
                            out=adj_i[:, 0:MMN], in_=adj[j * P :, 0:MMN]
                        )
                        nc.sync.dma_start(
                            out=adj_i[:, MMN:], in_=adj[j * P :, MMN:]
                        )
                    else:
                        nc.sync.dma_start(out=adj_i, in_=adj[j * P : (j + 1) * P, :])
                adj_b = bf_pool.tile([P, SLAB], mybir.dt.bfloat16)
                if last:
                    nc.vector.tensor_copy(adj_b[:, 0:MMN], adj_i[:, 0:MMN])
                    mm(wv(j), adj_b, 0, MMN)
                    nc.vector.tensor_copy(adj_b[:, MMN:], adj_i[:, MMN:])
                    mm(wv(j), adj_b, MMN, SLAB, stop=True)
                    continue
                nc.vector.tensor_copy(adj_b, adj_i)

                if j < TPC:
                    WL, WU = wv(j), wv(NT + j)
                    c0, c1 = j * P, (j + 1) * P
                    mlo = diag_pool.tile([P, P], mybir.dt.bfloat16)
                    nc.vector.tensor_mul(mlo, adj_b[:, c0:c1], msb[:, 0:P])
                    mup = diag_pool.tile([P, P], mybir.dt.bfloat16)
                    nc.vector.tensor_mul(mup, adj_b[:, c0:c1], msb[:, P : 2 * P])
                    # full columns left of the diag block: rows > cols -> U
                    mm(WU, adj_b, 0, c0)
                    mm_seg(WL, mlo, c0, c1)
                    mm_seg(WU, mup, c0, c1)
                    # full columns right of the diag block: rows < cols -> L
                    mm(WL, adj_b, c1, SLAB)
                else:
                    mm(wv(j), adj_b, 0, SLAB, stop=last)

            # per-bank copy-out: bank A's copy/DMA overlap the final bank-B
            # matmul (ACT reads psum bank A while PE writes bank B)
            out_sb = singles.tile([M, SLAB], mybir.dt.float32)
            nc.scalar.copy(out_sb[:, 0:MMN], accs[0])
            nc.sync.dma_start(out=stats[:, 0:MMN], in_=out_sb[:, 0:MMN])
            nc.scalar.copy(out_sb[:, MMN:], accs[1])
            nc.sync.dma_start(out=stats[:, MMN:], in_=out_sb[:, MMN:])

    nc.compile()
    return nc


def _split_bf16(v):
    hi = v.astype(BF16)
    lo = (v - hi.astype(np.float64)).astype(BF16)
    return hi, lo


def _host_prep(outputs, targets):
    """Per-row weight table Wside [N, 6] bf16 and per-core inputs."""
    out = np.asarray(outputs, np.float64).reshape(-1)
    pos = (np.asarray(targets).reshape(-1) != 0).astype(np.float64)
    pl_hi, pl_lo = _split_bf16(pos * out)
    e_hi, e_lo = _split_bf16(np.exp(out))
    wside = np.stack(
        [
            np.ones(N, BF16),
            pos.astype(BF16),
            pl_hi,
            pl_lo,
            e_hi,
            e_lo,
        ],
        axis=1,
    ).astype(BF16)  # [N, 6]

    # triangular masks for the diagonal 128-block (strict)
    ri = np.arange(P)[:, None]
    ci = np.arange(P)[None, :]
    masks = np.concatenate(
        [(ri < ci).astype(BF16), (ri > ci).astype(BF16)], axis=1
    )  # [128, 256]
    return wside, np.ascontiguousarray(masks)


def _build_wmat(wside, core):
    """Per-core weight variants [128, (64+8)*12] bf16.

    Variant j (j<64): weights for local row tile j (absolute tile (8*core+j)%64).
      j < 8  -> L-only variant (diag tiles; U-only twin stored at 64+j)
      j >= 8 -> single variant, L or U half per the tile's position vs the slab
    """
    w = np.zeros((P, NT + TPC, M), dtype=BF16)
    for j in range(NT):
        t = (TPC * core + j) % NT
        rows = wside[t * P : (t + 1) * P, :]  # [128, 6]
        if j < TPC:
            w[:, j, 0:NW] = rows
            w[:, NT + j, NW:M] = rows
        elif j < NT - TPC * core:
            w[:, j, NW:M] = rows  # rows above slab columns -> U
        else:
            w[:, j, 0:NW] = rows  # wrapped rows below slab columns -> L
    return np.ascontiguousarray(w.reshape(P, (NT + TPC) * M))


def _build_shard(node_adj, core):
    """Rotated column slab [N, SLAB] int32: local row rho = (abs_row - SLAB*core) mod N."""
    c0 = SLAB * core
    cols = node_adj[:, c0 : c0 + SLAB]
    if core == 0:
        return np.ascontiguousarray(cols, dtype=np.int32)
    return np.ascontiguousarray(
        np.concatenate([cols[c0:], cols[:c0]], axis=0), dtype=np.int32
    )


def _combine(stats_list, idx_node):
    """stats_list: per-core [12, SLAB] f32 -> scalar loss (f64 math)."""
    full = np.concatenate([np.asarray(s, np.float64) for s in stats_list], axis=1)

    def side_contrib(x):
        cnt, poscnt = x[0], x[1]
        poslogit = x[2] + x[3]
        sumexp = x[4] + x[5]
        valid = (cnt > 0.5) & (np.abs(poscnt - 1.0) < 0.25)
        lse = np.log(np.where(valid, np.maximum(sumexp, 1e-300), 1.0))
        return np.where(valid, (lse - poslogit) / np.maximum(cnt, 1.0), 0.0)

    contrib = side_contrib(full[0:NW]) + side_contrib(full[NW:M])
    idx = np.asarray(idx_node).reshape(-1).astype(np.int64)
    return np.array(contrib[idx].sum(), dtype=np.float32)


def _ensure_axon_hooks_stub():
    """bass_utils imports antenv.axon_hooks when tracing is requested via
    env; the module is absent on some images. Provide a no-op stub so the
    import never crashes (hook=None -> bass_utils skips tracing)."""
    import sys
    import types

    try:
        import antenv.axon_hooks  # noqa: F401
    except ImportError:
        mod = types.ModuleType("antenv.axon_hooks")
        state = {"hook": None}
        mod.set_axon_ntff_profile_hook = lambda h: state.__setitem__("hook", h)
        mod.get_axon_ntff_profile_hook = lambda: state["hook"]
        sys.modules["antenv.axon_hooks"] = mod


def _device_stats(in_maps):
    _ensure_axon_hooks_stub()
    from concourse.bass_utils import run_bass_kernel_spmd

    if "nc" not in _BASS_CACHE:
        _BASS_CACHE["nc"] = _build_bass()
    last_exc = None
    for attempt in range(4):
        try:
            res = run_bass_kernel_spmd(
                _BASS_CACHE["nc"], in_maps, core_ids=list(range(NCORES))
            )
            return [r["stats"] for r in res.results]
        except Exception as e:  # transient NRT/accelerator hiccups
            last_exc = e
            try:
                # a fresh PJRT client usually recovers a transiently
                # "unrecoverable" accelerator; mirrors a process restart
                import jax
                import jax.extend.backend as _jeb

                jax.clear_caches()
                _jeb.clear_backends()
            except Exception:
                pass
            import time

            time.sleep(2.0 * (attempt + 1))
    raise last_exc


def _sim_stats(in_maps):
    """Numpy emulation of the device kernel (same inputs), for logic validation."""
    outs = []
    for m in in_maps:
        adj = m["adj"].astype(np.float32)
        w = m["wmat"].reshape(P, NT + TPC, M).astype(np.float32)
        msk = m["masks"].astype(np.float32)
        lowm, upm = msk[:, 0:P], msk[:, P:]
        acc = np.zeros((M, SLAB), np.float32)
        for j in range(NT):
            tile = adj[j * P : (j + 1) * P, :]
            if j < TPC:
                WL, WU = w[:, j, :], w[:, NT + j, :]
                c0, c1 = j * P, (j + 1) * P
                acc[:, :c0] += WU.T @ tile[:, :c0]
                acc[:, c0:c1] += WL.T @ (tile[:, c0:c1] * lowm)
                acc[:, c0:c1] += WU.T @ (tile[:, c0:c1] * upm)
                acc[:, c1:] += WL.T @ tile[:, c1:]
            else:
                acc += w[:, j, :].T @ tile
        outs.append(acc)
    return outs


def kernel(outputs, targets, node_adj, idx_node, _simulate=False):
    node_adj = np.asarray(node_adj)
    wside, masks = _host_prep(outputs, targets)
    in_maps = [
        {
            "adj": _build_shard(node_adj, d),
            "wmat": _build_wmat(wside, d),
            "masks": masks,
        }
        for d in range(NCORES)
    ]
    stats = _sim_stats(in_maps) if _simulate else _device_stats(in_maps)
    return _combine(stats, idx_node)
